# revision 27
# baseline (speedup 1.0000x reference)
"""Causal relative multi-head attention (prefill) on 8 Trainium2 NeuronCores.

Reference computation (fp32):
    q = x @ Wq.T + bq ; k = x @ Wk.T + bk ; v = x @ Wv.T + bv      [B,S,D]
    p = pos @ Wp.T + bp                                            [S,D]
    scores = causal((q+p) @ k.T / sqrt(dk)) ; attn = softmax(scores)
    out = (attn @ v) @ Wo.T + bo                                   [B,S,D]
with B=4, S=2048, D=1024, H=16, dk=64.

Sharding: batch x head-group. Core c handles batch b=c//2 and head group
g=c%2 (8 heads = 512 of the 1024 qkv/concat dims). After attention, the
pair {2b, 2b+1} exchanges its bf16 attention outputs via a pairwise
AllGather (0.5 MB per 512-row chunk), then each core computes the FULL
out-projection for HALF the output columns (even core: cols 0:512, odd:
512:1024) from all 1024 concat dims. Host concatenates the column halves.
This replaces the fp32 partial-sum AllReduce (4x the wire traffic) of the
earlier revision.

On-device layouts (T = transposed, dims on partitions, seq on free axis):
    phase 1  q_posT/kT [512, S] and v [S, 512] from xposT (bf16 matmuls,
             p folded into q via host-concatenated [Wq|Wp]/[x;pos],
             scale+bias via DVE tensor_scalar).
    phase 2  per head-pair flash-style attention in scoresT layout
             [keys, queries]: row-packed K=64 score matmuls, additive
             triangular mask on diagonal blocks, ACT exp -> bf16,
             col-packed M=64 attn@v matmuls + rowsum-via-ones matmuls,
             approx-reciprocal normalize -> concat_oT bf16.
    phase 3  per 512-row chunk: DMA concat_oT chunk -> DRAM, pairwise
             AllGather, DMA both halves back, out half-columns =
             ccoT.T @ WoT_half + bias (host-replicated [128, 512] bias).

The whole thing is software-pipelined along seq: attention query-block
qb only needs projections for seq <= (qb+1)*512, and the exchange +
output projection run per 512-row chunk, so collective latency hides
under compute of later blocks.
"""

import numpy as np
import ml_dtypes

import concourse.bacc as bacc
import concourse.mybir as mybir
import concourse.tile as tile
from concourse.bass_utils import run_bass_kernel_spmd

F32R = mybir.dt.float32r
F32 = mybir.dt.float32
BF16 = mybir.dt.bfloat16
AFT = mybir.ActivationFunctionType
ALU = mybir.AluOpType

B, S, D = 4, 2048, 1024
H, DK = 16, 64
N_CORES = 8
GROUP_DIMS = 512              # qkv dims per head group (8 heads x 64)
OUT_COLS = 512                # output columns computed per core
SB = 512                      # phase-1 seq block
NSB = S // SB                 # 4
QB = 512                      # phase-2 query block / output chunk
NQB = S // QB                 # 4
NKT = S // 128                # 16 key tiles
MASK_NEG = -30000.0

_PROG = None
_last_in_maps = None


def _build_program():
    nc = bacc.Bacc("TRN2", target_bir_lowering=False, debug=False,
                   num_devices=N_CORES)

    xpos_d = nc.dram_tensor("xpos", [16, 128, S], BF16, kind="ExternalInput")
    wqp_d = nc.dram_tensor("wqp", [16, 128, GROUP_DIMS], BF16, kind="ExternalInput")
    wk_d = nc.dram_tensor("wk", [8, 128, GROUP_DIMS], BF16, kind="ExternalInput")
    wv_d = nc.dram_tensor("wv", [8, 128, GROUP_DIMS], BF16, kind="ExternalInput")
    wo_d = nc.dram_tensor("wo", [8, 128, OUT_COLS], BF16, kind="ExternalInput")
    bqp_d = nc.dram_tensor("bqp", [128, 4], F32, kind="ExternalInput")
    bk_d = nc.dram_tensor("bk", [128, 4], F32, kind="ExternalInput")
    bo_d = nc.dram_tensor("bo_bc", [128, OUT_COLS], F32, kind="ExternalInput")
    tri_d = nc.dram_tensor("tri", [128, 2, 128], F32, kind="ExternalInput")
    ones_d = nc.dram_tensor("ones64", [128, 64], BF16, kind="ExternalInput")
    y_d = nc.dram_tensor("y", [S, OUT_COLS], F32, kind="ExternalOutput")

    with tile.TileContext(nc) as tc:
        with (
            tc.tile_pool(name="wts", bufs=1) as wts,
            tc.tile_pool(name="xin", bufs=2) as xin,
            tc.tile_pool(name="big", bufs=1) as big,
            tc.tile_pool(name="att", bufs=4) as att,
            tc.tile_pool(name="rcp", bufs=2) as rcp,
            tc.tile_pool(name="qtp", bufs=2) as qtp,
            tc.tile_pool(name="outp", bufs=1) as outp,
            tc.tile_pool(name="cst", bufs=1) as cst,
            tc.tile_pool(name="ps", bufs=2, space="PSUM") as ps,
            tc.tile_pool(name="ps_s", bufs=2, space="PSUM") as ps_s,
            tc.tile_pool(name="ps_acc", bufs=1, space="PSUM") as ps_acc,
            tc.tile_pool(name="dram", bufs=1, space="DRAM") as dram,
        ):
            # ---- small constants first, on the scalar HWDGE ring (the
            # ---- sync ring carries the multi-MB weight/x streams and
            # ---- would delay these tiny-but-urgent transfers by ~30us)
            bqp_t = cst.tile([128, 4], F32)
            bk_t = cst.tile([128, 4], F32)
            tri_t = cst.tile([128, 2, 128], F32)
            bo_t = cst.tile([128, OUT_COLS], F32)
            wo_t = wts.tile([128, 8, OUT_COLS], BF16)
            nc.scalar.dma_start(bqp_t[:], bqp_d[:])
            nc.scalar.dma_start(bk_t[:], bk_d[:])
            nc.scalar.dma_start(tri_t[:], tri_d[:])
            nc.scalar.dma_start(bo_t[:], bo_d[:])
            for i in range(8):
                nc.scalar.dma_start(wo_t[:, i, :], wo_d[i])

            # ---- phase-1 weights + first x block, interleaved so the
            # ---- first q matmuls start after the first pair of DMAs ----
            wqp_t = wts.tile([128, 16, GROUP_DIMS], BF16)
            wk_t = wts.tile([128, 8, GROUP_DIMS], BF16)
            wv_t = wts.tile([128, 8, GROUP_DIMS], BF16)
            xp0_t = xin.tile([128, 16, SB], BF16, tag="xp")
            for i in range(16):
                nc.sync.dma_start(wqp_t[:, i, :], wqp_d[i])
                nc.sync.dma_start(xp0_t[:, i, :], xpos_d[i, :, 0:SB])
                if i < 8:
                    nc.sync.dma_start(wk_t[:, i, :], wk_d[i])
                    nc.sync.dma_start(wv_t[:, i, :], wv_d[i])

            kT = big.tile([128, 4, S], BF16)
            v_sb = big.tile([128, NKT, 4, 256], BF16)  # seq x [vA|1|vB|1] per pair
            nc.gpsimd.memset(v_sb[:, :, :, 64:128], 1.0)
            nc.gpsimd.memset(v_sb[:, :, :, 192:256], 1.0)
            qT_blocks = {}

            def phase1_block(sb):
                if sb == 0:
                    xp_t = xp0_t
                else:
                    # SWDGE (gpsimd) prefetch: keeps the sync HWDGE ring
                    # free for the latency-sensitive phase-2/3 transfers
                    xp_t = xin.tile([128, 16, SB], BF16, tag="xp")
                    for i in range(16):
                        nc.gpsimd.dma_start(xp_t[:, i, :],
                                            xpos_d[i, :, sb * SB:(sb + 1) * SB])
                qT = qtp.tile([128, 4, SB], BF16, tag="qT", name="qT")
                qT_blocks[sb] = qT
                # q/k interleaved per dt: head pair dt of the attention
                # block depends only on q/k chunk dt, so attention can
                # start as soon as the first dt is projected
                for dt in range(4):
                    psq = ps.tile([128, GROUP_DIMS], F32, tag="ps")
                    for i in range(16):
                        nc.tensor.matmul(psq[:, :SB],
                                         wqp_t[:, i, dt * 128:(dt + 1) * 128],
                                         xp_t[:, i, :],
                                         start=(i == 0), stop=(i == 15))
                    nc.vector.tensor_scalar(
                        qT[:, dt, :], psq[:, :SB],
                        0.125, bqp_t[:, dt:dt + 1],
                        op0=ALU.mult, op1=ALU.add)
                    psk = ps.tile([128, GROUP_DIMS], F32, tag="ps")
                    for i in range(8):
                        nc.tensor.matmul(psk[:, :SB],
                                         wk_t[:, i, dt * 128:(dt + 1) * 128],
                                         xp_t[:, i, :],
                                         start=(i == 0), stop=(i == 7))
                    nc.vector.tensor_scalar_add(
                        kT[:, dt, sb * SB:(sb + 1) * SB], psk[:, :SB],
                        bk_t[:, dt:dt + 1])
                for st in range(SB // 128):
                    psv = ps.tile([128, GROUP_DIMS], F32, tag="ps")
                    for i in range(8):
                        nc.tensor.matmul(psv[:],
                                         xp_t[:, i, st * 128:(st + 1) * 128],
                                         wv_t[:, i, :],
                                         start=(i == 0), stop=(i == 7))
                    pv = psv[:].rearrange("p (a b c) -> p a b c", a=4, b=2)
                    t = sb * 4 + st
                    dst = v_sb[:, t, :, :].rearrange("p a (b c) -> p a b c", b=2)
                    nc.vector.tensor_copy(dst[:, :, :, 0:64], pv[:])

            phase1_block(0)

            # per-chunk exchange buffers (distinct tiles -> no false deps),
            # split 3+1 head pairs so the final AllGather of each chunk
            # carries only one pair and hides behind almost no compute
            cci = [dram.tile([128, 3 if i % 2 == 0 else 1, QB], BF16,
                             tag=f"cci{i}", name=f"cci{i}")
                   for i in range(2 * NQB)]
            cco = [dram.tile([2, 128, 3 if i % 2 == 0 else 1, QB], BF16,
                             tag=f"cco{i}", name=f"cco{i}")
                   for i in range(2 * NQB)]
            RG = [[0, 1], [2, 3], [4, 5], [6, 7]]

            for qb in range(NQB):
                # ---- phase 2: attention for this query block, all head pairs
                qT = qT_blocks.pop(qb)
                for hp in range(4):
                    ps_oa = ps_acc.tile([128, QB], F32, tag="oa")
                    ps_ob = ps_acc.tile([128, QB], F32, tag="ob")
                    nkt = 4 * qb + 4
                    for kt in range(nkt):
                        d = kt - 4 * qb
                        n0 = max(0, 128 * d)
                        n1 = QB
                        s2 = ps_s.tile([128, 2, QB], F32, tag="s")
                        nc.tensor.matmul(s2[:, 0, n0:n1],
                                         kT[0:64, hp, kt * 128:(kt + 1) * 128],
                                         qT[0:64, hp, n0:n1],
                                         start=True, stop=True,
                                         tile_position=(0, 0))
                        nc.tensor.matmul(s2[:, 1, n0:n1],
                                         kT[64:128, hp, kt * 128:(kt + 1) * 128],
                                         qT[64:128, hp, n0:n1],
                                         start=True, stop=True,
                                         tile_position=(64, 0))
                        if d >= 0:
                            nc.vector.tensor_add(s2[:, :, n0:n0 + 128],
                                                 s2[:, :, n0:n0 + 128], tri_t[:])
                        e2 = att.tile([128, 2, QB], BF16, tag="exp")
                        nc.scalar.activation(e2[:, :, n0:n1], s2[:, :, n0:n1],
                                             AFT.Exp)
                        first = kt == 0
                        last = kt == nkt - 1
                        # fused attn@v + rowsum: stationary [vA|1] / [1|vB]
                        nc.tensor.matmul(ps_oa[:, n0:n1],
                                         v_sb[:, kt, hp, 0:128],
                                         e2[:, 0, n0:n1], start=first, stop=last)
                        nc.tensor.matmul(ps_ob[:, n0:n1],
                                         v_sb[:, kt, hp, 128:256],
                                         e2[:, 1, n0:n1], start=first, stop=last)
                    # DVE may only touch PSUM with full-height base-0 APs
                    # (base-64 PSUM reads corrupt SBUF); stage to SBUF first
                    # and do all partition-shifted work there.
                    # head A: o rows 0:64, rowsum rows 64:128 -> shift down
                    sta = rcp.tile([128, QB], F32, tag="sta")
                    rta = rcp.tile([64, QB], F32, tag="rta")
                    rca = rcp.tile([64, QB], F32, tag="rca")
                    oan = rcp.tile([64, QB], BF16, tag="oan")
                    nc.vector.tensor_copy(sta[:], ps_oa[:])
                    nc.vector.tensor_copy(rta[:], sta[64:128, :])
                    nc.vector.reciprocal_approx_fast(rca[:], rta[:])
                    nc.vector.tensor_mul(oan[:], sta[0:64, :], rca[:])
                    # head B: same layout; normalize at base 0
                    stb = rcp.tile([128, QB], F32, tag="stb")
                    rtb = rcp.tile([64, QB], F32, tag="rtb")
                    rcb = rcp.tile([64, QB], F32, tag="rcb")
                    obn = rcp.tile([64, QB], BF16, tag="obn")
                    nc.vector.tensor_copy(stb[:], ps_ob[:])
                    nc.vector.tensor_copy(rtb[:], stb[64:128, :])
                    nc.vector.reciprocal_approx_fast(rcb[:], rtb[:])
                    nc.vector.tensor_mul(obn[:], stb[0:64, :], rcb[:])
                    # stream straight into the exchange buffer (no SBUF
                    # staging tile; DMA moves partitions freely so head B
                    # lands on rows 64:128)
                    j, hw = (2 * qb, hp) if hp < 3 else (2 * qb + 1, 0)
                    nc.sync.dma_start(cci[j][0:64, hw, :], oan[:])
                    nc.sync.dma_start(cci[j][64:128, hw, :], obn[:])
                    if hp == 2:
                        nc.gpsimd.collective_compute(
                            "AllGather", mybir.AluOpType.bypass,
                            replica_groups=RG,
                            ins=[cci[2 * qb][:].opt()],
                            outs=[cco[2 * qb][:].opt()])
                    if hp == 3:
                        nc.gpsimd.collective_compute(
                            "AllGather", mybir.AluOpType.bypass,
                            replica_groups=RG,
                            ins=[cci[2 * qb + 1][:].opt()],
                            outs=[cco[2 * qb + 1][:].opt()])

                # ---- phase 1 for the next seq block (pipelined) ----
                if qb < NQB - 1:
                    phase1_block(qb + 1)

                # ---- phase 3: half-column out-proj for this chunk ----
                # co2 layout [128, rank*4 + hp, QB]
                co2 = outp.tile([128, 8, QB], BF16, tag="co2")
                for r in range(2):
                    nc.sync.dma_start(co2[:, r * 4:r * 4 + 3, :],
                                      cco[2 * qb][r])
                for r in range(2):
                    nc.sync.dma_start(co2[:, r * 4 + 3:r * 4 + 4, :],
                                      cco[2 * qb + 1][r])
                # accumulation order: 3-pair cks first, so the matmuls
                # start before the last single-pair exchange completes
                cks = [0, 1, 2, 4, 5, 6, 3, 7]
                for st in range(4):
                    sq = 4 * qb + st
                    pso = ps.tile([128, GROUP_DIMS], F32, tag="ps")
                    for j, ck in enumerate(cks):
                        nc.tensor.matmul(pso[:],
                                         co2[:, ck, st * 128:(st + 1) * 128],
                                         wo_t[:, ck, :],
                                         start=(j == 0), stop=(j == 7))
                    ot = outp.tile([128, OUT_COLS], F32, tag="out", bufs=2)
                    nc.vector.tensor_add(ot[:], pso[:], bo_t[:])
                    nc.sync.dma_start(y_d[sq * 128:(sq + 1) * 128, :], ot[:])

    nc.compile()
    return nc


def _get_program():
    global _PROG
    if _PROG is None:
        _PROG = _build_program()
    return _PROG


def kernel(x, pos_emb, Wq, bq, Wk, bk, Wv, bv, Wp, bp, Wo, bo):
    x = np.asarray(x, dtype=np.float32)
    pos_emb = np.asarray(pos_emb, dtype=np.float32)
    Wq, bq = np.asarray(Wq, np.float32), np.asarray(bq, np.float32)
    Wk, bk = np.asarray(Wk, np.float32), np.asarray(bk, np.float32)
    Wv, bv = np.asarray(Wv, np.float32), np.asarray(bv, np.float32)
    Wp, bp = np.asarray(Wp, np.float32), np.asarray(bp, np.float32)
    Wo, bo = np.asarray(Wo, np.float32), np.asarray(bo, np.float32)

    nc = _get_program()

    posT = np.ascontiguousarray(pos_emb.T)                      # [D, S]
    tri1 = np.where(np.arange(128)[:, None] <= np.arange(128)[None, :],
                    np.float32(0.0), np.float32(MASK_NEG)).astype(np.float32)
    tri = np.ascontiguousarray(
        np.broadcast_to(tri1[:, None, :], (128, 2, 128)))       # both heads
    ones64 = np.ones((128, 64), dtype=ml_dtypes.bfloat16)

    in_maps = []
    for c in range(N_CORES):
        b, g = divmod(c, 2)
        sl = slice(g * GROUP_DIMS, (g + 1) * GROUP_DIMS)        # qkv head dims
        so = slice(g * OUT_COLS, (g + 1) * OUT_COLS)            # out columns
        xT = np.ascontiguousarray(x[b].T)                       # [D, S]
        xpos = np.concatenate([xT, posT], axis=0).reshape(16, 128, S)
        wqpT = np.concatenate([Wq[sl].T, Wp[sl].T], axis=0)     # [2D, 512]
        wkT = np.ascontiguousarray(Wk[sl].T)                    # [D, 512]
        wvT = np.ascontiguousarray(Wv[sl].T)
        woT = np.ascontiguousarray(Wo[so, :].T)                 # [D, 512]
        bqp = ((bq[sl] + bp[sl]) * 0.125).reshape(4, 128).T     # [128, 4]
        bk2 = bk[sl].reshape(4, 128).T
        bo_eff = bo[so] + Wo[so, :] @ bv                        # [512]
        bo_bc = np.broadcast_to(bo_eff, (128, OUT_COLS))
        in_maps.append({
            "xpos": xpos.astype(ml_dtypes.bfloat16),
            "wqp": wqpT.reshape(16, 128, GROUP_DIMS).astype(ml_dtypes.bfloat16),
            "wk": wkT.reshape(8, 128, GROUP_DIMS).astype(ml_dtypes.bfloat16),
            "wv": wvT.reshape(8, 128, GROUP_DIMS).astype(ml_dtypes.bfloat16),
            "wo": woT.reshape(8, 128, OUT_COLS).astype(ml_dtypes.bfloat16),
            "bqp": np.ascontiguousarray(bqp, dtype=np.float32),
            "bk": np.ascontiguousarray(bk2, dtype=np.float32),
            "bo_bc": np.ascontiguousarray(bo_bc, dtype=np.float32),
            "tri": tri,
            "ones64": ones64,
        })

    global _last_in_maps
    _last_in_maps = in_maps

    res = run_bass_kernel_spmd(nc, in_maps, list(range(N_CORES)))
    out = np.stack(
        [np.concatenate([res.results[2 * b]["y"], res.results[2 * b + 1]["y"]],
                        axis=1)
         for b in range(B)], axis=0)
    return out.astype(np.float32)


# revision 31
# speedup vs baseline: 1.0475x; 1.0475x over previous
"""Causal relative multi-head attention (prefill) on 8 Trainium2 NeuronCores.

Reference computation (fp32):
    q = x @ Wq.T + bq ; k = x @ Wk.T + bk ; v = x @ Wv.T + bv      [B,S,D]
    p = pos @ Wp.T + bp                                            [S,D]
    scores = causal((q+p) @ k.T / sqrt(dk)) ; attn = softmax(scores)
    out = (attn @ v) @ Wo.T + bo                                   [B,S,D]
with B=4, S=2048, D=1024, H=16, dk=64.

Sharding: batch x head-group. Core c handles batch b=c//2 and head group
g=c%2 (8 heads = 512 of the 1024 qkv/concat dims). After attention, the
pair {2b, 2b+1} exchanges its bf16 attention outputs via pairwise
AllGathers (two 2-head-pair halves per 512-row chunk), then each core
computes the FULL out-projection for HALF the output columns (even core:
cols 0:512, odd: 512:1024) from all 1024 concat dims. Host concatenates
the column halves.

On-device layouts (T = transposed, dims on partitions, seq on free axis):
    phase 1  q_posT/kT [512, S] and v [S, 512] from xposT (bf16 matmuls,
             p folded into q via host-concatenated [Wq|Wp]/[x;pos],
             scale+bias via DVE tensor_scalar).
    phase 2  per head-pair flash-style attention in scoresT layout
             [keys, queries]: row-packed K=64 score matmuls, additive
             triangular mask on diagonal blocks, ACT exp -> bf16,
             col-packed M=64 attn@v matmuls + rowsum-via-ones matmuls,
             approx-reciprocal normalize, streamed into the exchange
             buffers per head pair.
    phase 3  per 512-row chunk: two pairwise AllGathers, DMA both ranks'
             halves back, out half-columns = ccoT.T @ WoT_half + bias.

Scheduling: the attention kt loop is software-pipelined (emit score(kt),
then a spliced chunk of phase-1 projections for the NEXT seq block or
phase-3 out-projection of the PREVIOUS chunk, then attn@v(kt-1)), so the
TensorE FIFO always holds work that is independent of the pending exp —
the PE never stalls on ScalarE and the HAM clock stays at 8/8.
"""

import numpy as np
import ml_dtypes

import concourse.bacc as bacc
import concourse.mybir as mybir
import concourse.tile as tile
from concourse.bass_utils import run_bass_kernel_spmd

F32R = mybir.dt.float32r
F32 = mybir.dt.float32
BF16 = mybir.dt.bfloat16
AFT = mybir.ActivationFunctionType
ALU = mybir.AluOpType

B, S, D = 4, 2048, 1024
H, DK = 16, 64
N_CORES = 8
GROUP_DIMS = 512              # qkv dims per head group (8 heads x 64)
OUT_COLS = 512                # output columns computed per core
SB = 512                      # phase-1 seq block
NSB = S // SB                 # 4
QB = 512                      # phase-2 query block / output chunk
NQB = S // QB                 # 4
NKT = S // 128                # 16 key tiles
MASK_NEG = -30000.0

_PROG = None
_last_in_maps = None


def _build_program():
    nc = bacc.Bacc("TRN2", target_bir_lowering=False, debug=False,
                   num_devices=N_CORES)

    xpos_d = nc.dram_tensor("xpos", [16, 128, S], BF16, kind="ExternalInput")
    wqp_d = nc.dram_tensor("wqp", [16, 128, GROUP_DIMS], BF16, kind="ExternalInput")
    wk_d = nc.dram_tensor("wk", [8, 128, GROUP_DIMS], BF16, kind="ExternalInput")
    wv_d = nc.dram_tensor("wv", [8, 128, GROUP_DIMS], BF16, kind="ExternalInput")
    wo_d = nc.dram_tensor("wo", [8, 128, OUT_COLS], BF16, kind="ExternalInput")
    bqp_d = nc.dram_tensor("bqp", [128, 4], F32, kind="ExternalInput")
    bk_d = nc.dram_tensor("bk", [128, 4], F32, kind="ExternalInput")
    bo_d = nc.dram_tensor("bo_bc", [128, OUT_COLS], F32, kind="ExternalInput")
    tri_d = nc.dram_tensor("tri", [128, 2, 128], F32, kind="ExternalInput")
    ones_d = nc.dram_tensor("ones64", [128, 64], BF16, kind="ExternalInput")
    y_d = nc.dram_tensor("y", [S, OUT_COLS], F32, kind="ExternalOutput")

    with tile.TileContext(nc) as tc:
        with (
            tc.tile_pool(name="wts", bufs=1) as wts,
            tc.tile_pool(name="xin", bufs=2) as xin,
            tc.tile_pool(name="big", bufs=1) as big,
            tc.tile_pool(name="att", bufs=4) as att,
            tc.tile_pool(name="rcp", bufs=2) as rcp,
            tc.tile_pool(name="qtp", bufs=2) as qtp,
            tc.tile_pool(name="outp", bufs=1) as outp,
            tc.tile_pool(name="cst", bufs=1) as cst,
            tc.tile_pool(name="ps", bufs=2, space="PSUM") as ps,
            tc.tile_pool(name="ps_s", bufs=2, space="PSUM") as ps_s,
            tc.tile_pool(name="ps_acc", bufs=1, space="PSUM") as ps_acc,
            tc.tile_pool(name="dram", bufs=1, space="DRAM") as dram,
        ):
            # ---- small constants first, on the scalar HWDGE ring (the
            # ---- sync ring carries the multi-MB weight/x streams and
            # ---- would delay these tiny-but-urgent transfers by ~30us)
            bqp_t = cst.tile([128, 4], F32)
            bk_t = cst.tile([128, 4], F32)
            tri_t = cst.tile([128, 2, 128], F32)
            bo_t = cst.tile([128, OUT_COLS], F32)
            wo_t = wts.tile([128, 8, OUT_COLS], BF16)
            nc.scalar.dma_start(bqp_t[:], bqp_d[:])
            nc.scalar.dma_start(bk_t[:], bk_d[:])
            nc.scalar.dma_start(tri_t[:], tri_d[:])
            nc.scalar.dma_start(bo_t[:], bo_d[:])
            for i in range(8):
                nc.scalar.dma_start(wo_t[:, i, :], wo_d[i])

            # ---- phase-1 weights + first x block, interleaved so the
            # ---- first q matmuls start after the first pair of DMAs ----
            wqp_t = wts.tile([128, 16, GROUP_DIMS], BF16)
            wk_t = wts.tile([128, 8, GROUP_DIMS], BF16)
            wv_t = wts.tile([128, 8, GROUP_DIMS], BF16)
            xp0_t = xin.tile([128, 16, SB], BF16, tag="xp")
            for i in range(16):
                nc.sync.dma_start(wqp_t[:, i, :], wqp_d[i])
                nc.sync.dma_start(xp0_t[:, i, :], xpos_d[i, :, 0:SB])
            for i in range(8):
                nc.sync.dma_start(wk_t[:, i, :], wk_d[i])
                nc.sync.dma_start(wv_t[:, i, :], wv_d[i])

            kT = big.tile([128, 4, S], BF16)
            v_sb = big.tile([128, NKT, 4, 256], BF16)  # seq x [vA|1|vB|1] per pair
            nc.gpsimd.memset(v_sb[:, :, :, 64:128], 1.0)
            nc.gpsimd.memset(v_sb[:, :, :, 192:256], 1.0)
            qT_blocks = {}

            def phase1_chunks(sb, xp_t):
                """Yield closures, each emitting a small group of phase-1
                matmuls (plus the DVE drain when a group completes)."""
                qT = qtp.tile([128, 4, SB], BF16, tag="qT", name="qT")
                qT_blocks[sb] = qT
                state = {}

                def q_half(dt, h):
                    def emit():
                        if h == 0:
                            state[dt] = ps.tile([128, GROUP_DIMS], F32,
                                                tag="ps", name="psq")
                        psq = state[dt]
                        for i in range(8 * h, 8 * h + 8):
                            nc.tensor.matmul(psq[:, :SB],
                                             wqp_t[:, i, dt * 128:(dt + 1) * 128],
                                             xp_t[:, i, :],
                                             start=(i == 0), stop=(i == 15))
                        if h == 1:
                            nc.vector.tensor_scalar(
                                qT[:, dt, :], psq[:, :SB],
                                0.125, bqp_t[:, dt:dt + 1],
                                op0=ALU.mult, op1=ALU.add)
                    return emit

                def k_grp(dt):
                    def emit():
                        psk = ps.tile([128, GROUP_DIMS], F32, tag="ps",
                                      name="psk")
                        for i in range(8):
                            nc.tensor.matmul(psk[:, :SB],
                                             wk_t[:, i, dt * 128:(dt + 1) * 128],
                                             xp_t[:, i, :],
                                             start=(i == 0), stop=(i == 7))
                        nc.vector.tensor_scalar_add(
                            kT[:, dt, sb * SB:(sb + 1) * SB], psk[:, :SB],
                            bk_t[:, dt:dt + 1])
                    return emit

                def v_grp(st):
                    def emit():
                        psv = ps.tile([128, GROUP_DIMS], F32, tag="ps",
                                      name="psv")
                        for i in range(8):
                            nc.tensor.matmul(psv[:],
                                             xp_t[:, i, st * 128:(st + 1) * 128],
                                             wv_t[:, i, :],
                                             start=(i == 0), stop=(i == 7))
                        pv = psv[:].rearrange("p (a b c) -> p a b c", a=4, b=2)
                        t = sb * 4 + st
                        dst = v_sb[:, t, :, :].rearrange(
                            "p a (b c) -> p a b c", b=2)
                        nc.vector.tensor_copy(dst[:, :, :, 0:64], pv[:])
                    return emit

                qchunks, kvchunks = [], []
                for dt in range(4):
                    qchunks.append(q_half(dt, 0))
                    qchunks.append(q_half(dt, 1))
                    kvchunks.append(k_grp(dt))
                    kvchunks.append(v_grp(dt))
                return qchunks, kvchunks

            def phase3_chunks(qb):
                """Yield closures for the out-projection of chunk qb
                (exchange already complete / in flight)."""
                co2 = outp.tile([128, 8, QB], BF16, tag="co2", name="co2")

                def readback():
                    for r in range(2):
                        nc.sync.dma_start(co2[:, r * 4:r * 4 + 2, :],
                                          cco[2 * qb][r])
                    for r in range(2):
                        nc.sync.dma_start(co2[:, r * 4 + 2:r * 4 + 4, :],
                                          cco[2 * qb + 1][r])

                def st_grp(st):
                    def emit():
                        if st == 0:
                            readback()
                        sq = 4 * qb + st
                        pso = ps.tile([128, GROUP_DIMS], F32, tag="ps",
                                      name="pso")
                        # first-half cks {0,1,4,5} first: those matmuls
                        # can start before the second exchange completes
                        for j, ck in enumerate([0, 1, 4, 5, 2, 3, 6, 7]):
                            nc.tensor.matmul(pso[:],
                                             co2[:, ck, st * 128:(st + 1) * 128],
                                             wo_t[:, ck, :],
                                             start=(j == 0), stop=(j == 7))
                        ot = outp.tile([128, OUT_COLS], F32, tag="out",
                                       bufs=2, name="ot")
                        nc.vector.tensor_add(ot[:], pso[:], bo_t[:])
                        nc.sync.dma_start(y_d[sq * 128:(sq + 1) * 128, :],
                                          ot[:])
                    return emit

                for st in range(4):
                    yield st_grp(st)

            # emit all of phase-1 for seq block 0 upfront (it is the only
            # PE work available at the start); interleave q/k/v per dt so
            # attention on head pair 0 can start after the first dt
            q0, kv0 = phase1_chunks(0, xp0_t)
            for dt in range(4):
                q0[2 * dt]()
                q0[2 * dt + 1]()
                kv0[2 * dt]()
                kv0[2 * dt + 1]()

            # per-half-chunk exchange buffers (distinct tiles -> no false
            # deps; halves let the exchange start after 2 of 4 head pairs)
            cci = [dram.tile([128, 2, QB], BF16, tag=f"cci{i}",
                             name=f"cci{i}") for i in range(2 * NQB)]
            cco = [dram.tile([2, 128, 2, QB], BF16, tag=f"cco{i}",
                             name=f"cco{i}") for i in range(2 * NQB)]
            RG = [[0, 1], [2, 3], [4, 5], [6, 7]]

            pending_kv = []
            for qb in range(NQB):
                # assemble the splice queue of independent matmul chunks
                # emitted between attention steps to keep the PE FIFO
                # dense: phase-1 of later seq blocks + phase-3 of earlier
                # chunks. Block 3's k/v and each chunk's out-projection
                # are deferred one block further so the long late
                # attention blocks also have filler work. p3 chunks go
                # last: their exchange may still be in flight early on.
                splices = []
                if qb < NQB - 1:
                    xp_t = xin.tile([128, 16, SB], BF16, tag="xp")
                    for i in range(16):
                        nc.gpsimd.dma_start(
                            xp_t[:, i, :],
                            xpos_d[i, :, (qb + 1) * SB:(qb + 2) * SB])
                    qc, kvc = phase1_chunks(qb + 1, xp_t)
                    if qb < 2:
                        splices = qc + kvc
                    else:
                        splices = qc
                        pending_kv = kvc
                else:
                    splices = pending_kv
                if qb > 0:
                    splices = splices + list(phase3_chunks(qb - 1))

                qT = qT_blocks.pop(qb)
                for hp in range(4):
                    ps_oa = ps_acc.tile([128, QB], F32, tag="oa")
                    ps_ob = ps_acc.tile([128, QB], F32, tag="ob")
                    nkt = 4 * qb + 4
                    pend = None  # (e2, n0, first, last)
                    for kt in range(nkt):
                        d = kt - 4 * qb
                        n0 = max(0, 128 * d)
                        n1 = QB
                        s2 = ps_s.tile([128, 2, QB], F32, tag="s")
                        nc.tensor.matmul(s2[:, 0, n0:n1],
                                         kT[0:64, hp, kt * 128:(kt + 1) * 128],
                                         qT[0:64, hp, n0:n1],
                                         start=True, stop=True,
                                         tile_position=(0, 0))
                        nc.tensor.matmul(s2[:, 1, n0:n1],
                                         kT[64:128, hp, kt * 128:(kt + 1) * 128],
                                         qT[64:128, hp, n0:n1],
                                         start=True, stop=True,
                                         tile_position=(64, 0))
                        if d >= 0:
                            nc.vector.tensor_add(s2[:, :, n0:n0 + 128],
                                                 s2[:, :, n0:n0 + 128],
                                                 tri_t[:])
                        e2 = att.tile([128, 2, QB], BF16, tag="exp")
                        nc.scalar.activation(e2[:, :, n0:n1], s2[:, :, n0:n1],
                                             AFT.Exp)
                        # spliced independent matmuls run while ScalarE
                        # computes exp(kt); then attn@v of the previous step
                        if splices:
                            splices.pop(0)()
                        if pend is not None:
                            pe2, pn0, pfirst = pend
                            nc.tensor.matmul(ps_oa[:, pn0:n1],
                                             v_sb[:, kt - 1, hp, 0:128],
                                             pe2[:, 0, pn0:n1],
                                             start=pfirst, stop=False)
                            nc.tensor.matmul(ps_ob[:, pn0:n1],
                                             v_sb[:, kt - 1, hp, 128:256],
                                             pe2[:, 1, pn0:n1],
                                             start=pfirst, stop=False)
                        pend = (e2, n0, kt == 0)
                    pe2, pn0, pfirst = pend
                    nc.tensor.matmul(ps_oa[:, pn0:QB],
                                     v_sb[:, nkt - 1, hp, 0:128],
                                     pe2[:, 0, pn0:QB],
                                     start=pfirst, stop=True)
                    nc.tensor.matmul(ps_ob[:, pn0:QB],
                                     v_sb[:, nkt - 1, hp, 128:256],
                                     pe2[:, 1, pn0:QB],
                                     start=pfirst, stop=True)
                    # DVE may only touch PSUM with full-height base-0 APs
                    # (base-64 PSUM reads corrupt SBUF); stage to SBUF
                    # first and do partition-shifted work there.
                    sta = rcp.tile([128, QB], F32, tag="sta")
                    rta = rcp.tile([64, QB], F32, tag="rta")
                    rca = rcp.tile([64, QB], F32, tag="rca")
                    oan = rcp.tile([64, QB], BF16, tag="oan")
                    nc.vector.tensor_copy(sta[:], ps_oa[:])
                    nc.vector.tensor_copy(rta[:], sta[64:128, :])
                    nc.vector.reciprocal_approx_fast(rca[:], rta[:])
                    nc.vector.tensor_mul(oan[:], sta[0:64, :], rca[:])
                    stb = rcp.tile([128, QB], F32, tag="stb")
                    rtb = rcp.tile([64, QB], F32, tag="rtb")
                    rcb = rcp.tile([64, QB], F32, tag="rcb")
                    obn = rcp.tile([64, QB], BF16, tag="obn")
                    nc.vector.tensor_copy(stb[:], ps_ob[:])
                    nc.vector.tensor_copy(rtb[:], stb[64:128, :])
                    nc.vector.reciprocal_approx_fast(rcb[:], rtb[:])
                    nc.vector.tensor_mul(obn[:], stb[0:64, :], rcb[:])
                    # stream straight into the exchange buffer (DMA moves
                    # partitions freely: head B lands on rows 64:128)
                    j, hw = 2 * qb + hp // 2, hp % 2
                    nc.sync.dma_start(cci[j][0:64, hw, :], oan[:])
                    nc.sync.dma_start(cci[j][64:128, hw, :], obn[:])
                    if hp == 1 or hp == 3:
                        nc.gpsimd.collective_compute(
                            "AllGather", mybir.AluOpType.bypass,
                            replica_groups=RG,
                            ins=[cci[j][:].opt()],
                            outs=[cco[j][:].opt()])

                # leftover splices (early chunks have more splice supply
                # than attention steps)
                for emit in splices:
                    emit()

            # tail: out-projection of the last chunk
            for emit in phase3_chunks(NQB - 1):
                emit()

    nc.compile()
    return nc


def _get_program():
    global _PROG
    if _PROG is None:
        _PROG = _build_program()
    return _PROG


def kernel(x, pos_emb, Wq, bq, Wk, bk, Wv, bv, Wp, bp, Wo, bo):
    x = np.asarray(x, dtype=np.float32)
    pos_emb = np.asarray(pos_emb, dtype=np.float32)
    Wq, bq = np.asarray(Wq, np.float32), np.asarray(bq, np.float32)
    Wk, bk = np.asarray(Wk, np.float32), np.asarray(bk, np.float32)
    Wv, bv = np.asarray(Wv, np.float32), np.asarray(bv, np.float32)
    Wp, bp = np.asarray(Wp, np.float32), np.asarray(bp, np.float32)
    Wo, bo = np.asarray(Wo, np.float32), np.asarray(bo, np.float32)

    nc = _get_program()

    posT = np.ascontiguousarray(pos_emb.T)                      # [D, S]
    tri1 = np.where(np.arange(128)[:, None] <= np.arange(128)[None, :],
                    np.float32(0.0), np.float32(MASK_NEG)).astype(np.float32)
    tri = np.ascontiguousarray(
        np.broadcast_to(tri1[:, None, :], (128, 2, 128)))       # both heads
    ones64 = np.ones((128, 64), dtype=ml_dtypes.bfloat16)

    in_maps = []
    for c in range(N_CORES):
        b, g = divmod(c, 2)
        sl = slice(g * GROUP_DIMS, (g + 1) * GROUP_DIMS)        # qkv head dims
        so = slice(g * OUT_COLS, (g + 1) * OUT_COLS)            # out columns
        xT = np.ascontiguousarray(x[b].T)                       # [D, S]
        xpos = np.concatenate([xT, posT], axis=0).reshape(16, 128, S)
        wqpT = np.concatenate([Wq[sl].T, Wp[sl].T], axis=0)     # [2D, 512]
        wkT = np.ascontiguousarray(Wk[sl].T)                    # [D, 512]
        wvT = np.ascontiguousarray(Wv[sl].T)
        woT = np.ascontiguousarray(Wo[so, :].T)                 # [D, 512]
        bqp = ((bq[sl] + bp[sl]) * 0.125).reshape(4, 128).T     # [128, 4]
        bk2 = bk[sl].reshape(4, 128).T
        bo_eff = bo[so] + Wo[so, :] @ bv                        # [512]
        bo_bc = np.broadcast_to(bo_eff, (128, OUT_COLS))
        in_maps.append({
            "xpos": xpos.astype(ml_dtypes.bfloat16),
            "wqp": wqpT.reshape(16, 128, GROUP_DIMS).astype(ml_dtypes.bfloat16),
            "wk": wkT.reshape(8, 128, GROUP_DIMS).astype(ml_dtypes.bfloat16),
            "wv": wvT.reshape(8, 128, GROUP_DIMS).astype(ml_dtypes.bfloat16),
            "wo": woT.reshape(8, 128, OUT_COLS).astype(ml_dtypes.bfloat16),
            "bqp": np.ascontiguousarray(bqp, dtype=np.float32),
            "bk": np.ascontiguousarray(bk2, dtype=np.float32),
            "bo_bc": np.ascontiguousarray(bo_bc, dtype=np.float32),
            "tri": tri,
            "ones64": ones64,
        })

    global _last_in_maps
    _last_in_maps = in_maps

    res = run_bass_kernel_spmd(nc, in_maps, list(range(N_CORES)))
    out = np.stack(
        [np.concatenate([res.results[2 * b]["y"], res.results[2 * b + 1]["y"]],
                        axis=1)
         for b in range(B)], axis=0)
    return out.astype(np.float32)


# revision 37
# speedup vs baseline: 1.0537x; 1.0059x over previous
"""Causal relative multi-head attention (prefill) on 8 Trainium2 NeuronCores.

Reference computation (fp32):
    q = x @ Wq.T + bq ; k = x @ Wk.T + bk ; v = x @ Wv.T + bv      [B,S,D]
    p = pos @ Wp.T + bp                                            [S,D]
    scores = causal((q+p) @ k.T / sqrt(dk)) ; attn = softmax(scores)
    out = (attn @ v) @ Wo.T + bo                                   [B,S,D]
with B=4, S=2048, D=1024, H=16, dk=64.

Sharding: batch x head-group. Core c handles batch b=c//2 and head group
g=c%2 (8 heads = 512 of the 1024 qkv/concat dims). After attention, the
pair {2b, 2b+1} exchanges its bf16 attention outputs via pairwise
AllGathers (two 2-head-pair halves per 512-row chunk), then each core
computes the FULL out-projection for HALF the output columns (even core:
cols 0:512, odd: 512:1024) from all 1024 concat dims. Host concatenates
the column halves.

On-device layouts (T = transposed, dims on partitions, seq on free axis):
    phase 1  q_posT/kT [512, S] and v [S, 512] from xposT (bf16 matmuls,
             p folded into q via host-concatenated [Wq|Wp]/[x;pos],
             scale+bias via DVE tensor_scalar).
    phase 2  per head-pair flash-style attention in scoresT layout
             [keys, queries]: row-packed K=64 score matmuls, additive
             triangular mask on diagonal blocks, ACT exp -> bf16,
             col-packed M=64 attn@v matmuls + rowsum-via-ones matmuls,
             approx-reciprocal normalize, streamed into the exchange
             buffers per head pair.
    phase 3  per 512-row chunk: two pairwise AllGathers, DMA both ranks'
             halves back, out half-columns = ccoT.T @ WoT_half + bias.

Scheduling: the attention kt loop is software-pipelined (emit score(kt),
then a spliced chunk of phase-1 projections for the NEXT seq block or
phase-3 out-projection of the PREVIOUS chunk, then attn@v(kt-1)), so the
TensorE FIFO always holds work that is independent of the pending exp —
the PE never stalls on ScalarE and the HAM clock stays at 8/8.
"""

import numpy as np
import ml_dtypes

import concourse.bacc as bacc
import concourse.mybir as mybir
import concourse.tile as tile
from concourse.bass_utils import run_bass_kernel_spmd

F32R = mybir.dt.float32r
F32 = mybir.dt.float32
BF16 = mybir.dt.bfloat16
AFT = mybir.ActivationFunctionType
ALU = mybir.AluOpType

B, S, D = 4, 2048, 1024
H, DK = 16, 64
N_CORES = 8
GROUP_DIMS = 512              # qkv dims per head group (8 heads x 64)
OUT_COLS = 512                # output columns computed per core
SB = 512                      # phase-1 seq block
NSB = S // SB                 # 4
QB = 512                      # phase-2 query block / output chunk
NQB = S // QB                 # 4
NKT = S // 128                # 16 key tiles
MASK_NEG = -30000.0

_PROG = None
_last_in_maps = None


def _build_program():
    nc = bacc.Bacc("TRN2", target_bir_lowering=False, debug=False,
                   num_devices=N_CORES)

    xpos_d = nc.dram_tensor("xpos", [16, 128, S], BF16, kind="ExternalInput")
    wqp_d = nc.dram_tensor("wqp", [16, 128, GROUP_DIMS], BF16, kind="ExternalInput")
    wk_d = nc.dram_tensor("wk", [8, 128, GROUP_DIMS], BF16, kind="ExternalInput")
    wv_d = nc.dram_tensor("wv", [8, 128, GROUP_DIMS], BF16, kind="ExternalInput")
    wo_d = nc.dram_tensor("wo", [8, 128, OUT_COLS], BF16, kind="ExternalInput")
    bqp_d = nc.dram_tensor("bqp", [128, 4], F32, kind="ExternalInput")
    bk_d = nc.dram_tensor("bk", [128, 4], F32, kind="ExternalInput")
    bo_d = nc.dram_tensor("bo_bc", [128, OUT_COLS], F32, kind="ExternalInput")
    tri_d = nc.dram_tensor("tri", [128, 2, 128], F32, kind="ExternalInput")
    ones_d = nc.dram_tensor("ones64", [128, 64], BF16, kind="ExternalInput")
    y_d = nc.dram_tensor("y", [S, OUT_COLS], F32, kind="ExternalOutput")

    with tile.TileContext(nc) as tc:
        with (
            tc.tile_pool(name="wts", bufs=1) as wts,
            tc.tile_pool(name="xin", bufs=2) as xin,
            tc.tile_pool(name="big", bufs=1) as big,
            tc.tile_pool(name="att", bufs=4) as att,
            tc.tile_pool(name="rcp", bufs=2) as rcp,
            tc.tile_pool(name="qtp", bufs=2) as qtp,
            tc.tile_pool(name="outp", bufs=1) as outp,
            tc.tile_pool(name="cst", bufs=1) as cst,
            tc.tile_pool(name="ps", bufs=2, space="PSUM") as ps,
            tc.tile_pool(name="ps_s", bufs=2, space="PSUM") as ps_s,
            tc.tile_pool(name="ps_acc", bufs=1, space="PSUM") as ps_acc,
            tc.tile_pool(name="dram", bufs=1, space="DRAM") as dram,
        ):
            # ---- small constants first, on the scalar HWDGE ring (the
            # ---- sync ring carries the multi-MB weight/x streams and
            # ---- would delay these tiny-but-urgent transfers by ~30us)
            bqp_t = cst.tile([128, 4], F32)
            bk_t = cst.tile([128, 4], F32)
            tri_t = cst.tile([128, 2, 128], F32)
            bo_t = cst.tile([128, OUT_COLS], F32)
            wo_t = wts.tile([128, 8, OUT_COLS], BF16)
            nc.scalar.dma_start(bqp_t[:], bqp_d[:])
            nc.scalar.dma_start(bk_t[:], bk_d[:])
            nc.scalar.dma_start(tri_t[:], tri_d[:])
            nc.scalar.dma_start(bo_t[:], bo_d[:])
            for i in range(8):
                nc.scalar.dma_start(wo_t[:, i, :], wo_d[i])

            # ---- phase-1 weights + first x block, interleaved so the
            # ---- first q matmuls start after the first pair of DMAs ----
            wqp_t = wts.tile([128, 16, GROUP_DIMS], BF16)
            wk_t = wts.tile([128, 8, GROUP_DIMS], BF16)
            wv_t = wts.tile([128, 8, GROUP_DIMS], BF16)
            xp0_t = xin.tile([128, 16, SB], BF16, tag="xp")
            for i in range(16):
                nc.sync.dma_start(wqp_t[:, i, :], wqp_d[i])
                nc.sync.dma_start(xp0_t[:, i, :], xpos_d[i, :, 0:SB])
            for i in range(8):
                nc.sync.dma_start(wk_t[:, i, :], wk_d[i])
                nc.sync.dma_start(wv_t[:, i, :], wv_d[i])

            kT = big.tile([128, 4, S], BF16)
            v_sb = big.tile([128, NKT, 4, 256], BF16)  # seq x [vA|1|vB|1] per pair
            nc.gpsimd.memset(v_sb[:, :, :, 64:128], 1.0)
            nc.gpsimd.memset(v_sb[:, :, :, 192:256], 1.0)
            qT_blocks = {}

            def phase1_chunks(sb, xp_t):
                """Yield closures, each emitting a small group of phase-1
                matmuls (plus the DVE drain when a group completes)."""
                qT = qtp.tile([128, 4, SB], BF16, tag="qT", name="qT")
                qT_blocks[sb] = qT
                state = {}

                def q_half(dt, h):
                    def emit():
                        if h == 0:
                            state[dt] = ps.tile([128, GROUP_DIMS], F32,
                                                tag="ps", name="psq")
                        psq = state[dt]
                        for i in range(8 * h, 8 * h + 8):
                            nc.tensor.matmul(psq[:, :SB],
                                             wqp_t[:, i, dt * 128:(dt + 1) * 128],
                                             xp_t[:, i, :],
                                             start=(i == 0), stop=(i == 15))
                        if h == 1:
                            nc.vector.tensor_scalar(
                                qT[:, dt, :], psq[:, :SB],
                                0.125, bqp_t[:, dt:dt + 1],
                                op0=ALU.mult, op1=ALU.add)
                    return emit

                def k_grp(dt):
                    def emit():
                        psk = ps.tile([128, GROUP_DIMS], F32, tag="ps",
                                      name="psk")
                        for i in range(8):
                            nc.tensor.matmul(psk[:, :SB],
                                             wk_t[:, i, dt * 128:(dt + 1) * 128],
                                             xp_t[:, i, :],
                                             start=(i == 0), stop=(i == 7))
                        nc.vector.tensor_scalar_add(
                            kT[:, dt, sb * SB:(sb + 1) * SB], psk[:, :SB],
                            bk_t[:, dt:dt + 1])
                    return emit

                def v_grp(st):
                    def emit():
                        psv = ps.tile([128, GROUP_DIMS], F32, tag="ps",
                                      name="psv")
                        for i in range(8):
                            nc.tensor.matmul(psv[:],
                                             xp_t[:, i, st * 128:(st + 1) * 128],
                                             wv_t[:, i, :],
                                             start=(i == 0), stop=(i == 7))
                        pv = psv[:].rearrange("p (a b c) -> p a b c", a=4, b=2)
                        t = sb * 4 + st
                        dst = v_sb[:, t, :, :].rearrange(
                            "p a (b c) -> p a b c", b=2)
                        nc.vector.tensor_copy(dst[:, :, :, 0:64], pv[:])
                    return emit

                qchunks, kvchunks = [], []
                for dt in range(4):
                    qchunks.append(q_half(dt, 0))
                    qchunks.append(q_half(dt, 1))
                    kvchunks.append(k_grp(dt))
                    kvchunks.append(v_grp(dt))
                return qchunks, kvchunks

            def phase3_chunks(qb):
                """Yield closures for the out-projection of chunk qb
                (exchange already complete / in flight)."""
                co2 = outp.tile([128, 8, QB], BF16, tag="co2", name="co2")

                def readback():
                    for r in range(2):
                        nc.sync.dma_start(co2[:, r * 4:r * 4 + 2, :],
                                          cco[2 * qb][r])
                    for r in range(2):
                        nc.sync.dma_start(co2[:, r * 4 + 2:r * 4 + 4, :],
                                          cco[2 * qb + 1][r])

                def st_grp(st):
                    def emit():
                        if st == 0:
                            readback()
                        sq = 4 * qb + st
                        pso = ps.tile([128, GROUP_DIMS], F32, tag="ps",
                                      name="pso")
                        # first-half cks {0,1,4,5} first: those matmuls
                        # can start before the second exchange completes
                        for j, ck in enumerate([0, 1, 4, 5, 2, 3, 6, 7]):
                            nc.tensor.matmul(pso[:],
                                             co2[:, ck, st * 128:(st + 1) * 128],
                                             wo_t[:, ck, :],
                                             start=(j == 0), stop=(j == 7))
                        ot = outp.tile([128, OUT_COLS], F32, tag="out",
                                       bufs=2, name="ot")
                        nc.vector.tensor_add(ot[:], pso[:], bo_t[:])
                        nc.sync.dma_start(y_d[sq * 128:(sq + 1) * 128, :],
                                          ot[:])
                    return emit

                for st in range(4):
                    yield st_grp(st)

            # emit all of phase-1 for seq block 0 upfront (it is the only
            # PE work available at the start); interleave q/k/v per dt so
            # attention on head pair 0 can start after the first dt
            q0, kv0 = phase1_chunks(0, xp0_t)
            for dt in range(4):
                q0[2 * dt]()
                q0[2 * dt + 1]()
                kv0[2 * dt]()
                kv0[2 * dt + 1]()

            # per-half-chunk exchange buffers (distinct tiles -> no false
            # deps; halves let the exchange start after 2 of 4 head pairs)
            cci = [dram.tile([128, 2, QB], BF16, tag=f"cci{i}",
                             name=f"cci{i}") for i in range(2 * NQB)]
            cco = [dram.tile([2, 128, 2, QB], BF16, tag=f"cco{i}",
                             name=f"cco{i}") for i in range(2 * NQB)]
            RG = [[0, 1], [2, 3], [4, 5], [6, 7]]

            pending_kv = []
            for qb in range(NQB):
                # assemble the splice queue of independent matmul chunks
                # emitted between attention steps to keep the PE FIFO
                # dense: phase-1 of later seq blocks + phase-3 of earlier
                # chunks. Block 3's k/v and each chunk's out-projection
                # are deferred one block further so the long late
                # attention blocks also have filler work. p3 chunks go
                # last: their exchange may still be in flight early on.
                splices = []
                if qb < NQB - 1:
                    xp_t = xin.tile([128, 16, SB], BF16, tag="xp")
                    for i in range(16):
                        nc.gpsimd.dma_start(
                            xp_t[:, i, :],
                            xpos_d[i, :, (qb + 1) * SB:(qb + 2) * SB])
                    qc, kvc = phase1_chunks(qb + 1, xp_t)
                    if qb < 2:
                        splices = qc + kvc
                    else:
                        splices = qc
                        pending_kv = kvc
                else:
                    splices = pending_kv
                if qb > 0:
                    splices = splices + list(phase3_chunks(qb - 1))

                qT = qT_blocks.pop(qb)
                for hp in range(4):
                    ps_oa = ps_acc.tile([128, QB], F32, tag="oa")
                    ps_ob = ps_acc.tile([128, QB], F32, tag="ob")
                    nkt = 4 * qb + 4
                    pend = None  # (e2, n0, first, last)
                    for kt in range(nkt):
                        d = kt - 4 * qb
                        n0 = max(0, 128 * d)
                        n1 = QB
                        s2 = ps_s.tile([128, 2, QB], F32, tag="s")
                        nc.tensor.matmul(s2[:, 0, n0:n1],
                                         kT[0:64, hp, kt * 128:(kt + 1) * 128],
                                         qT[0:64, hp, n0:n1],
                                         start=True, stop=True,
                                         tile_position=(0, 0))
                        nc.tensor.matmul(s2[:, 1, n0:n1],
                                         kT[64:128, hp, kt * 128:(kt + 1) * 128],
                                         qT[64:128, hp, n0:n1],
                                         start=True, stop=True,
                                         tile_position=(64, 0))
                        if d >= 0:
                            nc.vector.tensor_add(s2[:, :, n0:n0 + 128],
                                                 s2[:, :, n0:n0 + 128],
                                                 tri_t[:])
                        e2 = att.tile([128, 2, QB], BF16, tag="exp")
                        nc.scalar.activation(e2[:, :, n0:n1], s2[:, :, n0:n1],
                                             AFT.Exp)
                        # spliced independent matmuls run while ScalarE
                        # computes exp(kt); then attn@v of the previous step
                        if splices:
                            splices.pop(0)()
                        if pend is not None:
                            pe2, pn0, pfirst = pend
                            nc.tensor.matmul(ps_oa[:, pn0:n1],
                                             v_sb[:, kt - 1, hp, 0:128],
                                             pe2[:, 0, pn0:n1],
                                             start=pfirst, stop=False)
                            nc.tensor.matmul(ps_ob[:, pn0:n1],
                                             v_sb[:, kt - 1, hp, 128:256],
                                             pe2[:, 1, pn0:n1],
                                             start=pfirst, stop=False)
                        pend = (e2, n0, kt == 0)
                    pe2, pn0, pfirst = pend
                    nc.tensor.matmul(ps_oa[:, pn0:QB],
                                     v_sb[:, nkt - 1, hp, 0:128],
                                     pe2[:, 0, pn0:QB],
                                     start=pfirst, stop=True)
                    nc.tensor.matmul(ps_ob[:, pn0:QB],
                                     v_sb[:, nkt - 1, hp, 128:256],
                                     pe2[:, 1, pn0:QB],
                                     start=pfirst, stop=True)
                    # DVE may only touch PSUM with full-height base-0 APs
                    # (base-64 PSUM reads corrupt SBUF); stage to SBUF
                    # first and do partition-shifted work there.
                    sta = rcp.tile([128, QB], F32, tag="sta")
                    rta = rcp.tile([64, QB], F32, tag="rta")
                    rca = rcp.tile([64, QB], F32, tag="rca")
                    oan = rcp.tile([64, QB], BF16, tag="oan")
                    nc.vector.tensor_copy(sta[:], ps_oa[:])
                    nc.vector.tensor_copy(rta[:], sta[64:128, :])
                    nc.vector.reciprocal_approx_fast(rca[:], rta[:])
                    nc.vector.tensor_mul(oan[:], sta[0:64, :], rca[:])
                    stb = rcp.tile([128, QB], F32, tag="stb")
                    rtb = rcp.tile([64, QB], F32, tag="rtb")
                    rcb = rcp.tile([64, QB], F32, tag="rcb")
                    obn = rcp.tile([64, QB], BF16, tag="obn")
                    nc.vector.tensor_copy(stb[:], ps_ob[:])
                    nc.vector.tensor_copy(rtb[:], stb[64:128, :])
                    nc.vector.reciprocal_approx_fast(rcb[:], rtb[:])
                    nc.vector.tensor_mul(obn[:], stb[0:64, :], rcb[:])
                    # stream straight into the exchange buffer (DMA moves
                    # partitions freely: head B lands on rows 64:128)
                    j, hw = 2 * qb + hp // 2, hp % 2
                    nc.sync.dma_start(cci[j][0:64, hw, :], oan[:])
                    nc.sync.dma_start(cci[j][64:128, hw, :], obn[:])
                    if hp == 1 or hp == 3:
                        nc.gpsimd.collective_compute(
                            "AllGather", mybir.AluOpType.bypass,
                            replica_groups=RG,
                            ins=[cci[j][:].opt()],
                            outs=[cco[j][:].opt()])

                # leftover splices (early chunks have more splice supply
                # than attention steps)
                for emit in splices:
                    emit()

            # tail: out-projection of the last chunk
            for emit in phase3_chunks(NQB - 1):
                emit()

    nc.compile()
    return nc


def _get_program():
    global _PROG
    if _PROG is None:
        _PROG = _build_program()
    return _PROG


def kernel(x, pos_emb, Wq, bq, Wk, bk, Wv, bv, Wp, bp, Wo, bo):
    x = np.asarray(x, dtype=np.float32)
    pos_emb = np.asarray(pos_emb, dtype=np.float32)
    Wq, bq = np.asarray(Wq, np.float32), np.asarray(bq, np.float32)
    Wk, bk = np.asarray(Wk, np.float32), np.asarray(bk, np.float32)
    Wv, bv = np.asarray(Wv, np.float32), np.asarray(bv, np.float32)
    Wp, bp = np.asarray(Wp, np.float32), np.asarray(bp, np.float32)
    Wo, bo = np.asarray(Wo, np.float32), np.asarray(bo, np.float32)

    nc = _get_program()

    posT = np.ascontiguousarray(pos_emb.T)                      # [D, S]
    tri1 = np.where(np.arange(128)[:, None] <= np.arange(128)[None, :],
                    np.float32(0.0), np.float32(MASK_NEG)).astype(np.float32)
    tri = np.ascontiguousarray(
        np.broadcast_to(tri1[:, None, :], (128, 2, 128)))       # both heads
    ones64 = np.ones((128, 64), dtype=ml_dtypes.bfloat16)

    in_maps = []
    for c in range(N_CORES):
        b, g = divmod(c, 2)
        sl = slice(g * GROUP_DIMS, (g + 1) * GROUP_DIMS)        # qkv head dims
        so = slice(g * OUT_COLS, (g + 1) * OUT_COLS)            # out columns
        xT = np.ascontiguousarray(x[b].T)                       # [D, S]
        xpos = np.concatenate([xT, posT], axis=0).reshape(16, 128, S)
        wqpT = np.concatenate([Wq[sl].T, Wp[sl].T], axis=0)     # [2D, 512]
        wkT = np.ascontiguousarray(Wk[sl].T)                    # [D, 512]
        wvT = np.ascontiguousarray(Wv[sl].T)
        woT = np.ascontiguousarray(Wo[so, :].T)                 # [D, 512]
        bqp = ((bq[sl] + bp[sl]) * 0.125).reshape(4, 128).T     # [128, 4]
        bk2 = bk[sl].reshape(4, 128).T
        bo_eff = bo[so] + Wo[so, :] @ bv                        # [512]
        bo_bc = np.broadcast_to(bo_eff, (128, OUT_COLS))
        in_maps.append({
            "xpos": xpos.astype(ml_dtypes.bfloat16),
            "wqp": wqpT.reshape(16, 128, GROUP_DIMS).astype(ml_dtypes.bfloat16),
            "wk": wkT.reshape(8, 128, GROUP_DIMS).astype(ml_dtypes.bfloat16),
            "wv": wvT.reshape(8, 128, GROUP_DIMS).astype(ml_dtypes.bfloat16),
            "wo": woT.reshape(8, 128, OUT_COLS).astype(ml_dtypes.bfloat16),
            "bqp": np.ascontiguousarray(bqp, dtype=np.float32),
            "bk": np.ascontiguousarray(bk2, dtype=np.float32),
            "bo_bc": np.ascontiguousarray(bo_bc, dtype=np.float32),
            "tri": tri,
            "ones64": ones64,
        })

    global _last_in_maps
    _last_in_maps = in_maps

    res = run_bass_kernel_spmd(nc, in_maps, list(range(N_CORES)))
    out = np.stack(
        [np.concatenate([res.results[2 * b]["y"], res.results[2 * b + 1]["y"]],
                        axis=1)
         for b in range(B)], axis=0)
    return out.astype(np.float32)
